# revision 44
# baseline (speedup 1.0000x reference)
"""Trainium2 Bass kernel for masked multi-head attention with a rope-like
positional transform (nn_Attention_43937515438607).

Math per reference:
    qkv = x @ W_qkv.T + b_qkv                     (B,T,3C)
    q,k,v = split(qkv);  heads of D=64
    q = (q*pe0 + rot(q)*pe1) * pe2
    k = (k*pe0 + rot(k)*pe1) / pe2
    S = q k^T / sqrt(2D);  S[mask] = -inf;  alpha = softmax(S)
    out = alpha @ v  ->  (B,T,C)

Device strategy (8 cores, 2 batches per core):
  - projection as natural-layout fp16 matmuls (the PE's fp32r mode is
    ~10-bit mantissa anyway, and fp16 halves the x/W DMA); the qkv bias
    never hits
    TensorE: the rope'd Q/K bias tables (host-precomputed, exact) are added
    during the transpose-output move, and the V bias is applied after the
    softmax normalize (out = av/denom + bv)
  - rope: two multiplies on VectorE, final add on GpSimd (Pool). In the
    projection-only phase the PSUM read is staged through a ScalarE fp16
    copy so both VectorE multiplies run in the 2x 16-bit mode
  - Q,K transposed on TensorE (fp16 path, 1.0 cycles/row) into fp16 QT/KT;
    fp16 (not bf16) keeps the score mantissa error ~0.05%
  - softmax without max-subtraction: exp on ScalarE straight from PSUM;
    alpha/V/OT are bf16 because scores/TP reach +-33 and exp overflows
    fp16 range; mask applied as a bf16 multiply on VectorE (2x mode);
    denominator via a ones-column in the extended V operand
  - phase interleave: P1(b0) | P1(b1)+P3(b0) slot-interleaved |
    P3(b1)+final(b0) | final(b1,qc0) interleaved, final(b1,qc1) tail —
    keeps TensorE (167us busy) and ScalarE (exp, 133us) continuously fed
  - PSUM budget exactly 8 banks: proj(1) + tp(1) + scores(2x2) + oA/oB(2);
    the final-transpose pool opens only after the projection pools close
  - DMA choreography: x tile 0 + per-chunk weight/pe tiles first (TensorE
    starts ~4us in), mask/bias-table chunks slotted between steps

Measured: 199.5 us/core marginal HW time (41-iter loop delta), rel err
~6.1e-3 vs the fp32 reference. TimelineSim estimate 206.8 us.
"""

import sys

try:
    import concourse  # noqa: F401
except ImportError:  # pragma: no cover
    sys.path.insert(0, "/opt/trn_rl_repo")

import numpy as np
import ml_dtypes

from concourse import bass, mybir, tile, bacc
from concourse.bass_utils import run_bass_kernel_spmd
from concourse.masks import make_identity

# problem constants (hardcoded per harness contract)
B, T, C = 16, 1024, 512
NH = 8
D = C // NH
TP = float((2.0 * D) ** 0.5)
N_CORES = 8
BPC = B // N_CORES            # batches per core = 2
TOK = BPC * T                 # tokens per core  = 2048
NTT = TOK // 128              # token tiles per core = 16
NTB = T // 128                # token tiles per batch = 8
NHP = NH // 2                 # head pairs = 4
QC = 512                      # q chunk (PSUM bank) per attention unit
NQC = T // QC                 # q chunks per batch = 2

F32 = mybir.dt.float32
F32R = mybir.dt.float32r
BF16 = mybir.dt.bfloat16
FP16 = mybir.dt.float16


def build_nc(niter=1):
    nc = bacc.Bacc("TRN2", target_bir_lowering=False, debug=False)

    # ---- DRAM I/O ----
    xT_d = nc.dram_tensor("xT", [C, TOK], FP16, kind="ExternalInput")
    wT_d = nc.dram_tensor("wT", [C, 3 * C], FP16, kind="ExternalInput")
    pe_d = nc.dram_tensor("pe4", [4, T, D], FP16, kind="ExternalInput")
    tqT_d = nc.dram_tensor("tqT", [128, NHP, T], FP16, kind="ExternalInput")
    tkT_d = nc.dram_tensor("tkT", [128, NHP, T], FP16, kind="ExternalInput")
    bvb_d = nc.dram_tensor("bvb", [128, C], F32, kind="ExternalInput")
    nmT_d = nc.dram_tensor("nmT", [BPC, T, T], BF16, kind="ExternalInput")
    y_d = nc.dram_tensor("y", [TOK, C], F32, kind="ExternalOutput")

    VW = 66 * NH + 32            # V_ext row width = 560

    with tile.TileContext(nc) as tc:
        import contextlib
        loop_cm = tc.For_i(0, niter, 1) if niter > 1 else contextlib.nullcontext()
        ctx = contextlib.ExitStack()
        with loop_cm, ctx:
            persist = ctx.enter_context(tc.tile_pool(name="persist", bufs=1))
            V_sb = persist.tile([128, NTT, VW], BF16)
            QT = [persist.tile([128, NHP, T], FP16, tag=f"QT{b}", name=f"QT{b}")
                  for b in range(BPC)]
            KT = [persist.tile([128, NHP, T], FP16, tag=f"KT{b}", name=f"KT{b}")
                  for b in range(BPC)]
            OT = [persist.tile([96, NH, T], BF16, tag=f"OT{b}", name=f"OT{b}")
                  for b in range(BPC)]
            id_hf = persist.tile([128, 128], FP16)
            id_bf = persist.tile([128, 128], BF16)

            make_identity(nc, id_hf[:])
            make_identity(nc, id_bf[:])
            # only the pad columns need zeroing (V data cols are written by
            # the V copies; cols 558:560 are never read)
            nc.vector.memset(V_sb[:, :, 65::66], 0.0)
            nc.vector.memset(V_sb[:, :, 528:558], 0.0)
            nc.vector.memset(V_sb[:, :, 64::66], 1.0)

            # mask tiles; DMA chunks are interleaved into the P1(b0) steps
            # below so they don't delay the weight/x transfers at startup
            mT = [persist.tile([128, NTB, T], BF16, tag=f"mT{b}", name=f"mT{b}")
                  for b in range(BPC)]

            def mask_dma(chunk):
                b, kg = chunk // 4, chunk % 4
                nc.sync.dma_start(
                    mT[b][:, kg * 2:(kg + 1) * 2, :],
                    nmT_d[b][kg * 256:(kg + 1) * 256, :].rearrange(
                        "(kt p) q -> p kt q", p=128))

            # attention pools are created after scope A below (so P1(b0) can
            # use the PSUM banks they will occupy); p3_group is only called
            # once they exist.
            s_ps = o_ps = alpha_pool = None

            def p3_group(b, hp, qc):
                """One attention group; generator yields after each kt."""
                hA, hB = 2 * hp, 2 * hp + 1
                oA = o_ps.tile([96, QC], F32, tag="oA", name="oA")
                oB = o_ps.tile([96, QC], F32, tag="oB", name="oB")

                def emit_av(al, kt):
                    vbase = b * NTB + kt
                    nc.tensor.matmul(
                        oA[:],
                        V_sb[:, vbase, hA * 66:hA * 66 + 96],
                        al[:, 0:QC],
                        start=(kt == 0), stop=(kt == NTB - 1))
                    nc.tensor.matmul(
                        oB[:],
                        V_sb[:, vbase, hB * 66:hB * 66 + 96],
                        al[:, QC:2 * QC],
                        start=(kt == 0), stop=(kt == NTB - 1))

                pend = []
                for kt in range(NTB):
                    sp = s_ps.tile([128, 2 * QC], F32, tag="s")
                    nc.tensor.matmul(
                        sp[:, 0:QC],
                        KT[b][0:64, hp, kt * 128:(kt + 1) * 128],
                        QT[b][0:64, hp, qc * QC:(qc + 1) * QC],
                        start=True, stop=True)
                    nc.tensor.matmul(
                        sp[:, QC:2 * QC],
                        KT[b][64:128, hp, kt * 128:(kt + 1) * 128],
                        QT[b][64:128, hp, qc * QC:(qc + 1) * QC],
                        start=True, stop=True)
                    al = alpha_pool.tile([128, 2 * QC], BF16, tag="al")
                    nc.scalar.activation(
                        al[:], sp[:],
                        mybir.ActivationFunctionType.Exp,
                        scale=1.0 / TP)
                    nc.vector.tensor_tensor(
                        al[:].rearrange("p (h q) -> p h q", h=2),
                        al[:].rearrange("p (h q) -> p h q", h=2),
                        mT[b][:, kt, qc * QC:(qc + 1) * QC][:, None, :]
                        .to_broadcast([128, 2, QC]),
                        mybir.AluOpType.mult)
                    pend.append((al, kt))
                    if len(pend) > 2:
                        emit_av(*pend.pop(0))
                    yield
                for p in pend:
                    emit_av(*p)
                for oo, hh in ((oA, hA), (oB, hB)):
                    nc.vector.tensor_copy(
                        OT[b][:, hh, qc * QC:(qc + 1) * QC], oo[:])
                yield

            groups = [(b, hp, qc) for b in range(BPC)
                      for hp in range(NHP) for qc in range(NQC)]

            # ---------- phase 1 machinery (pools passed per scope) ----
            projin = ctx.enter_context(tc.tile_pool(name="projin", bufs=1))
            xin_pool = ctx.enter_context(tc.tile_pool(name="xin", bufs=4))
            qkn_pool = ctx.enter_context(tc.tile_pool(name="qkn", bufs=3))
            ropet = ctx.enter_context(tc.tile_pool(name="ropet", bufs=3))

            def p1_x(tt):
                xt = xin_pool.tile([128, 4, 128], FP16, tag="xt")
                nc.sync.dma_start(
                    xt[:],
                    xT_d[:, tt * 128:(tt + 1) * 128].rearrange(
                        "(ko p) t -> p ko t", p=128))
                return xt

            # x(tt0) first so TensorE can start ~3us in; wT and pe as
            # per-chunk tiles so consumers wait only on their own chunk
            wTk = [projin.tile([128, 3 * C], FP16, tag=f"wT{k}", name=f"wT{k}")
                   for k in range(4)]
            peAll = projin.tile([128, 4, NTB, D], FP16)
            peA = [peAll[:, 0], peAll[:, 2]]
            peB = [peAll[:, 1], peAll[:, 3]]
            biasT = [projin.tile([128, NHP, T], FP16, tag=f"bT{f}",
                                 name=f"bT{f}") for f in range(2)]
            bvb = projin.tile([128, C], F32)

            xts = {0: p1_x(0)}
            for ko in range(4):
                nc.sync.dma_start(wTk[ko][:], wT_d[ko * 128:(ko + 1) * 128, :])
            xts[1] = p1_x(1)
            nc.sync.dma_start(
                peAll[:], pe_d.rearrange("f (tb p) d -> p f tb d", p=128))
            xts[2] = p1_x(2)
            nc.sync.dma_start(biasT[0][:], tqT_d[:])
            nc.sync.dma_start(biasT[1][:], tkT_d[:])

            pend_tp = []

            def emit_tp(tp_ps, qk, fc, b, ttb):
                tp = tp_ps.tile([128, 512], FP16, tag="tp", name="tp")
                for hp in range(NHP):
                    nc.tensor.matmul(
                        tp[:, hp * 128:(hp + 1) * 128],
                        qk[:, hp * 128:(hp + 1) * 128],
                        id_hf[:],
                        is_transpose=True)
                dstt = (QT if fc == 0 else KT)[b]
                nc.vector.tensor_tensor(
                    dstt[:, :, ttb * 128:(ttb + 1) * 128],
                    tp[:].rearrange("p (hp t) -> p hp t", hp=NHP),
                    biasT[fc][:, :, ttb * 128:(ttb + 1) * 128],
                    mybir.AluOpType.add)

            def queue_tp(tp_ps, item):
                # depth-2 queue: the transpose runs two fc-slots after its
                # rope, past the DVE->Pool add latency
                pend_tp.append(item)
                if len(pend_tp) > 2:
                    emit_tp(tp_ps, *pend_tp.pop(0))

            def flush_tp(tp_ps):
                while pend_tp:
                    emit_tp(tp_ps, *pend_tp.pop(0))

            def p1_fc(proj_ps, tp_ps, tt, fc, xt, act_rope=False):
                b, ttb = tt // NTB, tt % NTB
                ps = proj_ps.tile([128, 512], F32, tag="proj")
                for kk in range(4):
                    nc.tensor.matmul(
                        ps[:], xt[:, kk, :],
                        wTk[kk][:, fc * 512:(fc + 1) * 512],
                        start=(kk == 0), stop=(kk == 3))

                if fc < 2:  # Q or K: rope -> fp16, then transpose
                    A = peA[fc][:, ttb, :]
                    Bp = peB[fc][:, ttb, :]
                    qk = qkn_pool.tile([128, C], FP16, tag=f"qk{fc}",
                                       name=f"qk{fc}")
                    if act_rope:
                        # ScalarE idles in this phase: stage PSUM->fp16 SBUF
                        # there so both DVE multiplies run in 2x mode
                        sb = ropet.tile([128, C], FP16, tag="sb")
                        nc.scalar.copy(sb[:], ps[:])
                        src3 = sb[:].rearrange("p (h d) -> p h d", h=NH)
                        src4 = sb[:].rearrange(
                            "p (h x two) -> p h x two", h=NH, two=2)
                        tdt = FP16
                    else:
                        src3 = ps[:].rearrange("p (h d) -> p h d", h=NH)
                        src4 = ps[:].rearrange(
                            "p (h x two) -> p h x two", h=NH, two=2)
                        tdt = F32
                    t1 = ropet.tile([128, NH, D], tdt, tag="t1")
                    nc.vector.tensor_tensor(
                        t1[:], src3,
                        A[:, None, :].to_broadcast([128, NH, D]),
                        mybir.AluOpType.mult)
                    t2 = ropet.tile([128, NH, D], tdt, tag="t2")
                    nc.vector.tensor_tensor(
                        t2[:].rearrange("p h (x two) -> p h x two", two=2),
                        src4[:, :, :, ::-1],
                        Bp[:, None, :].rearrange(
                            "p o (x two) -> p o x two", two=2
                        ).to_broadcast([128, NH, D // 2, 2]),
                        mybir.AluOpType.mult)
                    nc.gpsimd.tensor_tensor(
                        qk[:].rearrange("p (h d) -> p h d", h=NH),
                        t1[:], t2[:], mybir.AluOpType.add)
                    queue_tp(tp_ps, (qk, fc, b, ttb))
                else:  # V
                    vdst = V_sb[:, tt, :528].rearrange(
                        "p (h e) -> p h e", h=NH)[:, :, :D]
                    vsrc = ps[:].rearrange("p (h d) -> p h d", h=NH)
                    nc.scalar.copy(vdst, vsrc)

            # ---- scope A: P1 for batch 0 alone, deep PSUM buffering
            # (attention PSUM pools are not yet allocated)
            with tc.tile_pool(name="proj_psA", bufs=3, space="PSUM") as ppsA, \
                 tc.tile_pool(name="tp_psA", bufs=2, space="PSUM") as tpsA:
                for tt in range(NTB):
                    xt = xts.pop(tt) if tt in xts else p1_x(tt)
                    for fc in range(3):
                        p1_fc(ppsA, tpsA, tt, fc, xt, act_rope=True)
                    if tt >= 4:
                        mask_dma(tt - 4)   # mT[0] chunks
                flush_tp(tpsA)

            # ---- attention pools (live from here to the end of the iter)
            s_ps = ctx.enter_context(
                tc.tile_pool(name="s_ps", bufs=2, space="PSUM"))
            o_ps = ctx.enter_context(
                tc.tile_pool(name="o_ps", bufs=1, space="PSUM"))
            alpha_pool = ctx.enter_context(
                tc.tile_pool(name="alpha", bufs=6))

            # ---- scope B: P1(b1) interleaved with P3(b0).  proj/tp get one
            # bank each; >=2 kt of attention work sits between consecutive
            # p1_fc emissions so TensorE never waits on the rope chain.
            with tc.tile_pool(name="proj_psB", bufs=1, space="PSUM") as ppsB, \
                 tc.tile_pool(name="tp_psB", bufs=1, space="PSUM") as tpsB:
                for i, tt in enumerate(range(NTB, NTT)):
                    xt = p1_x(tt)
                    if i < 4:
                        mask_dma(4 + i)    # mT[1] chunks
                    elif i == 4:
                        nc.sync.dma_start(bvb[:], bvb_d[:])
                    g = p3_group(*groups[i])
                    next(g)            # kt0
                    next(g)            # kt1
                    p1_fc(ppsB, tpsB, tt, 0, xt)
                    next(g)            # kt2
                    next(g)            # kt3
                    next(g)            # kt4
                    p1_fc(ppsB, tpsB, tt, 1, xt)
                    for _ in g:        # kt5..kt7, tail + OT copies
                        pass
                    p1_fc(ppsB, tpsB, tt, 2, xt)
                flush_tp(tpsB)

            # ---------- tail: P3(b1) interleaved with final(b0) ----------
            with tc.tile_pool(name="fin_ps", bufs=1, space="PSUM") as fin_ps, \
                 tc.tile_pool(name="fin_sb", bufs=3) as fin_sb:

                def final_tile(b, qt, endgame=False):
                    out_sb = fin_sb.tile([128, C], F32, tag="out")
                    for half in range(2):
                        fp = fin_ps.tile([128, 4 * 96], BF16, tag=f"fin{half}",
                                         name=f"fin{half}")
                        for hh in range(4):
                            h = half * 4 + hh
                            nc.tensor.matmul(
                                fp[:, hh * 96:(hh + 1) * 96],
                                OT[b][:, h, qt * 128:(qt + 1) * 128],
                                id_bf[0:96, 0:96],
                                is_transpose=True)
                        rc = fin_sb.tile([128, 4], F32, tag=f"rc{half}",
                                         name=f"rc{half}")
                        nc.vector.reciprocal(rc[:], fp[:, 64::96])
                        nc.vector.tensor_tensor(
                            out_sb[:, half * 256:(half + 1) * 256].rearrange(
                                "p (h d) -> p h d", h=4),
                            fp[:].rearrange("p (h e) -> p h e", e=96)[:, :, :D],
                            rc[:][:, :, None].to_broadcast([128, 4, D]),
                            mybir.AluOpType.mult)
                        eng = nc.vector if endgame else nc.gpsimd
                        eng.tensor_tensor(
                            out_sb[:, half * 256:(half + 1) * 256],
                            out_sb[:, half * 256:(half + 1) * 256],
                            bvb[:, half * 256:(half + 1) * 256],
                            mybir.AluOpType.add)
                        row = b * T + qt * 128
                        nc.sync.dma_start(
                            y_d[row:row + 128, half * 256:(half + 1) * 256],
                            out_sb[:, half * 256:(half + 1) * 256])


                groups_b1 = [(1, hp, qc) for qc in range(NQC)
                                 for hp in range(NHP)]
                for i, grp in enumerate(groups_b1):
                    for _ in p3_group(*grp):
                        pass
                    final_tile(0, i)
                    if i >= 4:     # qc0 of b1 complete after 4 groups
                        final_tile(1, i - 4)
                for qt in range(4, NTB):
                    final_tile(1, qt, endgame=True)

    nc.compile()
    return nc


_NC_CACHE = None


def _get_nc():
    global _NC_CACHE
    if _NC_CACHE is None:
        _NC_CACHE = build_nc()
    return _NC_CACHE


def prep_inputs(x, pe0, pe1, pe2, mask, W_qkv, b_qkv):
    """Host-side layout prep + per-core sharding. Returns list of in_maps."""
    x = np.asarray(x, dtype=np.float32)
    pe0 = np.asarray(pe0, dtype=np.float32).reshape(T, D)
    pe1 = np.asarray(pe1, dtype=np.float32).reshape(T, D)
    pe2 = np.asarray(pe2, dtype=np.float32).reshape(T, D)
    mask = np.asarray(mask).astype(bool).reshape(B, T, T)
    W_qkv = np.asarray(W_qkv, dtype=np.float32)
    b_qkv = np.asarray(b_qkv, dtype=np.float32)

    wT = np.ascontiguousarray(W_qkv.T).astype(np.float16)  # [C, 3C]

    # rope tables: q' = q*A + swap(q)*B' ; A=pe0*pe2, B=pe1*pe2 (sign-folded)
    Aq = pe0 * pe2
    Bq = pe1 * pe2
    Ak = pe0 / pe2
    Bk = pe1 / pe2
    sign = np.ones((T, D), dtype=np.float32)
    sign[:, 0::2] = -1.0
    pe4 = np.ascontiguousarray(
        np.stack([Aq, Bq * sign, Ak, Bk * sign], axis=0)).astype(np.float16)

    # rope'd bias tables, pre-transposed to the QT/KT layout
    # Tq[t,h,d] = bq[h,d]*A[t,d] + bq[h,swap(d)]*(B*sign)[t,d]
    bq, bk, bv = (b_qkv[0:C].reshape(NH, D), b_qkv[C:2 * C].reshape(NH, D),
                  b_qkv[2 * C:3 * C])
    def rope_bias(bias_hd, A, Bs):
        bsw = bias_hd.reshape(NH, D // 2, 2)[:, :, ::-1].reshape(NH, D)
        Tt = (bias_hd[None] * A[:, None, :]
              + bsw[None] * Bs[:, None, :])                 # [T, NH, D]
        TT = Tt.reshape(T, NHP, 2, D).transpose(2, 3, 1, 0)  # [2, D, NHP, T]
        return np.ascontiguousarray(TT.reshape(128, NHP, T)
                                    .astype(np.float16))
    tqT = rope_bias(bq, Aq, Bq * sign)
    tkT = rope_bias(bk, Ak, Bk * sign)
    bvb = np.ascontiguousarray(
        np.broadcast_to(bv[None, :], (128, C)).astype(np.float32))

    notmask = (~mask).astype(ml_dtypes.bfloat16)            # [B,T,T] {0,1}
    in_maps = []
    for c in range(N_CORES):
        bs = slice(c * BPC, (c + 1) * BPC)
        xc = np.ascontiguousarray(
            x[bs].reshape(TOK, C).T).astype(np.float16)      # [C, TOK]
        nmT = np.ascontiguousarray(
            notmask[bs].transpose(0, 2, 1))                  # [BPC, T(kv), T(q)]
        in_maps.append(dict(
            xT=xc, wT=wT, pe4=pe4, nmT=nmT,
            tqT=tqT, tkT=tkT, bvb=bvb,
        ))
    return in_maps


def assemble_output(results):
    out = np.empty((B, T, C), dtype=np.float32)
    for c in range(N_CORES):
        out[c * BPC:(c + 1) * BPC] = results[c]["y"].reshape(BPC, T, C)
    return out


def kernel(x, pe0, pe1, pe2, mask, W_qkv, b_qkv):
    nc = _get_nc()
    in_maps = prep_inputs(x, pe0, pe1, pe2, mask, W_qkv, b_qkv)
    res = run_bass_kernel_spmd(nc, in_maps, core_ids=list(range(N_CORES)))
    return assemble_output(res.results)


# revision 45
# speedup vs baseline: 1.0535x; 1.0535x over previous
"""Trainium2 Bass kernel for masked multi-head attention with a rope-like
positional transform (nn_Attention_43937515438607).

Math per reference:
    qkv = x @ W_qkv.T + b_qkv                     (B,T,3C)
    q,k,v = split(qkv);  heads of D=64
    q = (q*pe0 + rot(q)*pe1) * pe2
    k = (k*pe0 + rot(k)*pe1) / pe2
    S = q k^T / sqrt(2D);  S[mask] = -inf;  alpha = softmax(S)
    out = alpha @ v  ->  (B,T,C)

Device strategy (8 cores, 2 batches per core):
  - projection as natural-layout fp16 matmuls (the PE's fp32r mode is
    ~10-bit mantissa anyway, and fp16 halves the x/W DMA); the qkv bias
    never hits
    TensorE: the rope'd Q/K bias tables (host-precomputed, exact) are added
    during the transpose-output move, and the V bias is applied after the
    softmax normalize (out = av/denom + bv)
  - rope: two multiplies on VectorE, final add on GpSimd (Pool). In the
    projection-only phase the PSUM read is staged through a ScalarE fp16
    copy so both VectorE multiplies run in the 2x 16-bit mode
  - Q,K transposed on TensorE (fp16 path, 1.0 cycles/row) into fp16 QT/KT;
    fp16 (not bf16) keeps the score mantissa error ~0.05%
  - softmax without max-subtraction: exp on ScalarE straight from PSUM;
    alpha/V/OT are bf16 because scores/TP reach +-33 and exp overflows
    fp16 range; mask applied as a bf16 multiply on VectorE (2x mode);
    denominator via a ones-column in the extended V operand
  - phase interleave: P1(b0) | P1(b1)+P3(b0) slot-interleaved |
    P3(b1)+final(b0) | final(b1,qc0) interleaved, final(b1,qc1) tail —
    keeps TensorE (167us busy) and ScalarE (exp, 133us) continuously fed
  - PSUM budget exactly 8 banks: proj(1) + tp(1) + scores(2x2) + oA/oB(2);
    the final-transpose pool opens only after the projection pools close
  - DMA choreography: x tile 0 + per-chunk weight/pe tiles first (TensorE
    starts ~4us in), mask/bias-table chunks slotted between steps

Measured: 199.5 us/core marginal HW time (41-iter loop delta), rel err
~6.1e-3 vs the fp32 reference. TimelineSim estimate 206.8 us.
"""

import sys

try:
    import concourse  # noqa: F401
except ImportError:  # pragma: no cover
    sys.path.insert(0, "/opt/trn_rl_repo")

import numpy as np
import ml_dtypes

from concourse import bass, mybir, tile, bacc
from concourse.bass_utils import run_bass_kernel_spmd
from concourse.masks import make_identity

# problem constants (hardcoded per harness contract)
B, T, C = 16, 1024, 512
NH = 8
D = C // NH
TP = float((2.0 * D) ** 0.5)
N_CORES = 8
BPC = B // N_CORES            # batches per core = 2
TOK = BPC * T                 # tokens per core  = 2048
NTT = TOK // 128              # token tiles per core = 16
NTB = T // 128                # token tiles per batch = 8
NHP = NH // 2                 # head pairs = 4
QC = 512                      # q chunk (PSUM bank) per attention unit
NQC = T // QC                 # q chunks per batch = 2

F32 = mybir.dt.float32
F32R = mybir.dt.float32r
BF16 = mybir.dt.bfloat16
FP16 = mybir.dt.float16


def build_nc(niter=1):
    nc = bacc.Bacc("TRN2", target_bir_lowering=False, debug=False)

    # ---- DRAM I/O ----
    xT_d = nc.dram_tensor("xT", [C, TOK], FP16, kind="ExternalInput")
    wT_d = nc.dram_tensor("wT", [C, 3 * C], FP16, kind="ExternalInput")
    pe_d = nc.dram_tensor("pe4", [4, T, D], FP16, kind="ExternalInput")
    tqT_d = nc.dram_tensor("tqT", [128, NHP, T], FP16, kind="ExternalInput")
    tkT_d = nc.dram_tensor("tkT", [128, NHP, T], FP16, kind="ExternalInput")
    bvb_d = nc.dram_tensor("bvb", [128, C], F32, kind="ExternalInput")
    nmT_d = nc.dram_tensor("nmT", [BPC, T, T], BF16, kind="ExternalInput")
    y_d = nc.dram_tensor("y", [TOK, C], F32, kind="ExternalOutput")

    VW = 66 * NH + 32            # V_ext row width = 560

    with tile.TileContext(nc) as tc:
        import contextlib
        loop_cm = tc.For_i(0, niter, 1) if niter > 1 else contextlib.nullcontext()
        ctx = contextlib.ExitStack()
        with loop_cm, ctx:
            persist = ctx.enter_context(tc.tile_pool(name="persist", bufs=1))
            V_sb = persist.tile([128, NTT, VW], BF16)
            QT = [persist.tile([128, NHP, T], FP16, tag=f"QT{b}", name=f"QT{b}")
                  for b in range(BPC)]
            KT = [persist.tile([128, NHP, T], FP16, tag=f"KT{b}", name=f"KT{b}")
                  for b in range(BPC)]
            OT = [persist.tile([96, NH, T], BF16, tag=f"OT{b}", name=f"OT{b}")
                  for b in range(BPC)]
            id_hf = persist.tile([128, 128], FP16)
            id_bf = persist.tile([128, 128], BF16)

            make_identity(nc, id_hf[:])
            make_identity(nc, id_bf[:])
            # only the pad columns need zeroing (V data cols are written by
            # the V copies; cols 558:560 are never read)
            nc.vector.memset(V_sb[:, :, 65::66], 0.0)
            nc.vector.memset(V_sb[:, :, 528:558], 0.0)
            nc.vector.memset(V_sb[:, :, 64::66], 1.0)

            # mask tiles; DMA chunks are interleaved into the P1(b0) steps
            # below so they don't delay the weight/x transfers at startup
            mT = [persist.tile([128, NTB, T], BF16, tag=f"mT{b}", name=f"mT{b}")
                  for b in range(BPC)]

            def mask_dma(chunk):
                b, kg = chunk // 4, chunk % 4
                nc.sync.dma_start(
                    mT[b][:, kg * 2:(kg + 1) * 2, :],
                    nmT_d[b][kg * 256:(kg + 1) * 256, :].rearrange(
                        "(kt p) q -> p kt q", p=128))

            # attention pools are created after scope A below (so P1(b0) can
            # use the PSUM banks they will occupy); p3_group is only called
            # once they exist.
            s_ps = o_ps = alpha_pool = None

            def p3_group(b, hp, qc):
                """One attention group; generator yields after each kt."""
                hA, hB = 2 * hp, 2 * hp + 1
                oA = o_ps.tile([96, QC], F32, tag="oA", name="oA")
                oB = o_ps.tile([96, QC], F32, tag="oB", name="oB")

                def emit_av(al, kt):
                    vbase = b * NTB + kt
                    nc.tensor.matmul(
                        oA[:],
                        V_sb[:, vbase, hA * 66:hA * 66 + 96],
                        al[:, 0:QC],
                        start=(kt == 0), stop=(kt == NTB - 1))
                    nc.tensor.matmul(
                        oB[:],
                        V_sb[:, vbase, hB * 66:hB * 66 + 96],
                        al[:, QC:2 * QC],
                        start=(kt == 0), stop=(kt == NTB - 1))

                pend = []
                for kt in range(NTB):
                    sp = s_ps.tile([128, 2 * QC], F32, tag="s")
                    nc.tensor.matmul(
                        sp[:, 0:QC],
                        KT[b][0:64, hp, kt * 128:(kt + 1) * 128],
                        QT[b][0:64, hp, qc * QC:(qc + 1) * QC],
                        start=True, stop=True)
                    nc.tensor.matmul(
                        sp[:, QC:2 * QC],
                        KT[b][64:128, hp, kt * 128:(kt + 1) * 128],
                        QT[b][64:128, hp, qc * QC:(qc + 1) * QC],
                        start=True, stop=True)
                    al = alpha_pool.tile([128, 2 * QC], BF16, tag="al")
                    nc.scalar.activation(
                        al[:], sp[:],
                        mybir.ActivationFunctionType.Exp,
                        scale=1.0 / TP)
                    nc.vector.tensor_tensor(
                        al[:].rearrange("p (h q) -> p h q", h=2),
                        al[:].rearrange("p (h q) -> p h q", h=2),
                        mT[b][:, kt, qc * QC:(qc + 1) * QC][:, None, :]
                        .to_broadcast([128, 2, QC]),
                        mybir.AluOpType.mult)
                    pend.append((al, kt))
                    if len(pend) > 2:
                        emit_av(*pend.pop(0))
                    yield
                for p in pend:
                    emit_av(*p)
                for oo, hh in ((oA, hA), (oB, hB)):
                    nc.vector.tensor_copy(
                        OT[b][:, hh, qc * QC:(qc + 1) * QC], oo[:])
                yield

            groups = [(b, hp, qc) for b in range(BPC)
                      for hp in range(NHP) for qc in range(NQC)]

            # ---------- phase 1 machinery (pools passed per scope) ----
            projin = ctx.enter_context(tc.tile_pool(name="projin", bufs=1))
            xin_pool = ctx.enter_context(tc.tile_pool(name="xin", bufs=4))
            qkn_pool = ctx.enter_context(tc.tile_pool(name="qkn", bufs=3))
            ropet = ctx.enter_context(tc.tile_pool(name="ropet", bufs=3))

            def p1_x(tt):
                xt = xin_pool.tile([128, 4, 128], FP16, tag="xt")
                nc.sync.dma_start(
                    xt[:],
                    xT_d[:, tt * 128:(tt + 1) * 128].rearrange(
                        "(ko p) t -> p ko t", p=128))
                return xt

            # x(tt0) first so TensorE can start ~3us in; wT and pe as
            # per-chunk tiles so consumers wait only on their own chunk
            wTk = [projin.tile([128, 3 * C], FP16, tag=f"wT{k}", name=f"wT{k}")
                   for k in range(4)]
            peF = [projin.tile([128, NTB, D], FP16, tag=f"pe{f}", name=f"pe{f}")
                   for f in range(4)]
            peA = [peF[0], peF[2]]
            peB = [peF[1], peF[3]]
            biasT = [projin.tile([128, NHP, T], FP16, tag=f"bT{f}",
                                 name=f"bT{f}") for f in range(2)]
            bvb = projin.tile([128, C], F32)

            xts = {0: p1_x(0)}
            # fc-major wT chunks: the first projection needs only the four
            # fc0 slices, not the full weight matrix
            for fc in range(3):
                for ko in range(4):
                    nc.sync.dma_start(
                        wTk[ko][:, fc * 512:(fc + 1) * 512],
                        wT_d[ko * 128:(ko + 1) * 128,
                             fc * 512:(fc + 1) * 512])
            xts[1] = p1_x(1)
            for f in range(4):
                nc.sync.dma_start(
                    peF[f][:], pe_d[f].rearrange("(tb p) d -> p tb d", p=128))
            xts[2] = p1_x(2)
            nc.sync.dma_start(biasT[0][:], tqT_d[:])
            nc.sync.dma_start(biasT[1][:], tkT_d[:])

            pend_tp = []

            def emit_tp(tp_ps, qk, fc, b, ttb):
                tp = tp_ps.tile([128, 512], FP16, tag="tp", name="tp")
                for hp in range(NHP):
                    nc.tensor.matmul(
                        tp[:, hp * 128:(hp + 1) * 128],
                        qk[:, hp * 128:(hp + 1) * 128],
                        id_hf[:],
                        is_transpose=True)
                dstt = (QT if fc == 0 else KT)[b]
                nc.vector.tensor_tensor(
                    dstt[:, :, ttb * 128:(ttb + 1) * 128],
                    tp[:].rearrange("p (hp t) -> p hp t", hp=NHP),
                    biasT[fc][:, :, ttb * 128:(ttb + 1) * 128],
                    mybir.AluOpType.add)

            def queue_tp(tp_ps, item):
                # depth-2 queue: the transpose runs two fc-slots after its
                # rope, past the DVE->Pool add latency
                pend_tp.append(item)
                if len(pend_tp) > 2:
                    emit_tp(tp_ps, *pend_tp.pop(0))

            def flush_tp(tp_ps):
                while pend_tp:
                    emit_tp(tp_ps, *pend_tp.pop(0))

            def p1_fc(proj_ps, tp_ps, tt, fc, xt, act_rope=False):
                b, ttb = tt // NTB, tt % NTB
                ps = proj_ps.tile([128, 512], F32, tag="proj")
                for kk in range(4):
                    nc.tensor.matmul(
                        ps[:], xt[:, kk, :],
                        wTk[kk][:, fc * 512:(fc + 1) * 512],
                        start=(kk == 0), stop=(kk == 3))

                if fc < 2:  # Q or K: rope -> fp16, then transpose
                    A = peA[fc][:, ttb, :]
                    Bp = peB[fc][:, ttb, :]
                    qk = qkn_pool.tile([128, C], FP16, tag=f"qk{fc}",
                                       name=f"qk{fc}")
                    if act_rope:
                        # ScalarE idles in this phase: stage PSUM->fp16 SBUF
                        # there so both DVE multiplies run in 2x mode
                        sb = ropet.tile([128, C], FP16, tag="sb")
                        nc.scalar.copy(sb[:], ps[:])
                        src3 = sb[:].rearrange("p (h d) -> p h d", h=NH)
                        src4 = sb[:].rearrange(
                            "p (h x two) -> p h x two", h=NH, two=2)
                        tdt = FP16
                    else:
                        src3 = ps[:].rearrange("p (h d) -> p h d", h=NH)
                        src4 = ps[:].rearrange(
                            "p (h x two) -> p h x two", h=NH, two=2)
                        tdt = F32
                    t1 = ropet.tile([128, NH, D], tdt, tag="t1")
                    nc.vector.tensor_tensor(
                        t1[:], src3,
                        A[:, None, :].to_broadcast([128, NH, D]),
                        mybir.AluOpType.mult)
                    t2 = ropet.tile([128, NH, D], tdt, tag="t2")
                    nc.vector.tensor_tensor(
                        t2[:].rearrange("p h (x two) -> p h x two", two=2),
                        src4[:, :, :, ::-1],
                        Bp[:, None, :].rearrange(
                            "p o (x two) -> p o x two", two=2
                        ).to_broadcast([128, NH, D // 2, 2]),
                        mybir.AluOpType.mult)
                    nc.gpsimd.tensor_tensor(
                        qk[:].rearrange("p (h d) -> p h d", h=NH),
                        t1[:], t2[:], mybir.AluOpType.add)
                    queue_tp(tp_ps, (qk, fc, b, ttb))
                else:  # V
                    vdst = V_sb[:, tt, :528].rearrange(
                        "p (h e) -> p h e", h=NH)[:, :, :D]
                    vsrc = ps[:].rearrange("p (h d) -> p h d", h=NH)
                    nc.scalar.copy(vdst, vsrc)

            # ---- scope A: P1 for batch 0 alone, deep PSUM buffering
            # (attention PSUM pools are not yet allocated)
            with tc.tile_pool(name="proj_psA", bufs=3, space="PSUM") as ppsA, \
                 tc.tile_pool(name="tp_psA", bufs=2, space="PSUM") as tpsA:
                for tt in range(NTB):
                    xt = xts.pop(tt) if tt in xts else p1_x(tt)
                    for fc in range(3):
                        p1_fc(ppsA, tpsA, tt, fc, xt, act_rope=True)
                    if tt >= 4:
                        mask_dma(tt - 4)   # mT[0] chunks
                flush_tp(tpsA)

            # ---- attention pools (live from here to the end of the iter)
            s_ps = ctx.enter_context(
                tc.tile_pool(name="s_ps", bufs=2, space="PSUM"))
            o_ps = ctx.enter_context(
                tc.tile_pool(name="o_ps", bufs=1, space="PSUM"))
            alpha_pool = ctx.enter_context(
                tc.tile_pool(name="alpha", bufs=6))

            # ---- scope B: P1(b1) interleaved with P3(b0).  proj/tp get one
            # bank each; >=2 kt of attention work sits between consecutive
            # p1_fc emissions so TensorE never waits on the rope chain.
            with tc.tile_pool(name="proj_psB", bufs=1, space="PSUM") as ppsB, \
                 tc.tile_pool(name="tp_psB", bufs=1, space="PSUM") as tpsB:
                for i, tt in enumerate(range(NTB, NTT)):
                    xt = p1_x(tt)
                    if i < 4:
                        mask_dma(4 + i)    # mT[1] chunks
                    elif i == 4:
                        nc.sync.dma_start(bvb[:], bvb_d[:])
                    g = p3_group(*groups[i])
                    next(g)            # kt0
                    next(g)            # kt1
                    p1_fc(ppsB, tpsB, tt, 0, xt)
                    next(g)            # kt2
                    next(g)            # kt3
                    next(g)            # kt4
                    p1_fc(ppsB, tpsB, tt, 1, xt)
                    for _ in g:        # kt5..kt7, tail + OT copies
                        pass
                    p1_fc(ppsB, tpsB, tt, 2, xt)
                flush_tp(tpsB)

            # ---------- tail: P3(b1) interleaved with final(b0) ----------
            with tc.tile_pool(name="fin_ps", bufs=1, space="PSUM") as fin_ps, \
                 tc.tile_pool(name="fin_sb", bufs=3) as fin_sb:

                def final_tile(b, qt, endgame=False):
                    out_sb = fin_sb.tile([128, C], F32, tag="out")
                    for half in range(2):
                        fp = fin_ps.tile([128, 4 * 96], BF16, tag=f"fin{half}",
                                         name=f"fin{half}")
                        for hh in range(4):
                            h = half * 4 + hh
                            nc.tensor.matmul(
                                fp[:, hh * 96:(hh + 1) * 96],
                                OT[b][:, h, qt * 128:(qt + 1) * 128],
                                id_bf[0:96, 0:96],
                                is_transpose=True)
                        rc = fin_sb.tile([128, 4], F32, tag=f"rc{half}",
                                         name=f"rc{half}")
                        nc.vector.reciprocal(rc[:], fp[:, 64::96])
                        nc.vector.tensor_tensor(
                            out_sb[:, half * 256:(half + 1) * 256].rearrange(
                                "p (h d) -> p h d", h=4),
                            fp[:].rearrange("p (h e) -> p h e", e=96)[:, :, :D],
                            rc[:][:, :, None].to_broadcast([128, 4, D]),
                            mybir.AluOpType.mult)
                        eng = nc.vector if endgame else nc.gpsimd
                        eng.tensor_tensor(
                            out_sb[:, half * 256:(half + 1) * 256],
                            out_sb[:, half * 256:(half + 1) * 256],
                            bvb[:, half * 256:(half + 1) * 256],
                            mybir.AluOpType.add)
                        row = b * T + qt * 128
                        nc.sync.dma_start(
                            y_d[row:row + 128, half * 256:(half + 1) * 256],
                            out_sb[:, half * 256:(half + 1) * 256])


                groups_b1 = [(1, hp, qc) for qc in range(NQC)
                                 for hp in range(NHP)]
                for i, grp in enumerate(groups_b1):
                    for _ in p3_group(*grp):
                        pass
                    final_tile(0, i)
                    if i >= 4:     # qc0 of b1 complete after 4 groups
                        final_tile(1, i - 4)
                for qt in range(4, NTB):
                    final_tile(1, qt, endgame=True)

    nc.compile()
    return nc


_NC_CACHE = None


def _get_nc():
    global _NC_CACHE
    if _NC_CACHE is None:
        _NC_CACHE = build_nc()
    return _NC_CACHE


def prep_inputs(x, pe0, pe1, pe2, mask, W_qkv, b_qkv):
    """Host-side layout prep + per-core sharding. Returns list of in_maps."""
    x = np.asarray(x, dtype=np.float32)
    pe0 = np.asarray(pe0, dtype=np.float32).reshape(T, D)
    pe1 = np.asarray(pe1, dtype=np.float32).reshape(T, D)
    pe2 = np.asarray(pe2, dtype=np.float32).reshape(T, D)
    mask = np.asarray(mask).astype(bool).reshape(B, T, T)
    W_qkv = np.asarray(W_qkv, dtype=np.float32)
    b_qkv = np.asarray(b_qkv, dtype=np.float32)

    wT = np.ascontiguousarray(W_qkv.T).astype(np.float16)  # [C, 3C]

    # rope tables: q' = q*A + swap(q)*B' ; A=pe0*pe2, B=pe1*pe2 (sign-folded)
    Aq = pe0 * pe2
    Bq = pe1 * pe2
    Ak = pe0 / pe2
    Bk = pe1 / pe2
    sign = np.ones((T, D), dtype=np.float32)
    sign[:, 0::2] = -1.0
    pe4 = np.ascontiguousarray(
        np.stack([Aq, Bq * sign, Ak, Bk * sign], axis=0)).astype(np.float16)

    # rope'd bias tables, pre-transposed to the QT/KT layout
    # Tq[t,h,d] = bq[h,d]*A[t,d] + bq[h,swap(d)]*(B*sign)[t,d]
    bq, bk, bv = (b_qkv[0:C].reshape(NH, D), b_qkv[C:2 * C].reshape(NH, D),
                  b_qkv[2 * C:3 * C])
    def rope_bias(bias_hd, A, Bs):
        bsw = bias_hd.reshape(NH, D // 2, 2)[:, :, ::-1].reshape(NH, D)
        Tt = (bias_hd[None] * A[:, None, :]
              + bsw[None] * Bs[:, None, :])                 # [T, NH, D]
        TT = Tt.reshape(T, NHP, 2, D).transpose(2, 3, 1, 0)  # [2, D, NHP, T]
        return np.ascontiguousarray(TT.reshape(128, NHP, T)
                                    .astype(np.float16))
    tqT = rope_bias(bq, Aq, Bq * sign)
    tkT = rope_bias(bk, Ak, Bk * sign)
    bvb = np.ascontiguousarray(
        np.broadcast_to(bv[None, :], (128, C)).astype(np.float32))

    notmask = (~mask).astype(ml_dtypes.bfloat16)            # [B,T,T] {0,1}
    in_maps = []
    for c in range(N_CORES):
        bs = slice(c * BPC, (c + 1) * BPC)
        xc = np.ascontiguousarray(
            x[bs].reshape(TOK, C).T).astype(np.float16)      # [C, TOK]
        nmT = np.ascontiguousarray(
            notmask[bs].transpose(0, 2, 1))                  # [BPC, T(kv), T(q)]
        in_maps.append(dict(
            xT=xc, wT=wT, pe4=pe4, nmT=nmT,
            tqT=tqT, tkT=tkT, bvb=bvb,
        ))
    return in_maps


def assemble_output(results):
    out = np.empty((B, T, C), dtype=np.float32)
    for c in range(N_CORES):
        out[c * BPC:(c + 1) * BPC] = results[c]["y"].reshape(BPC, T, C)
    return out


def kernel(x, pe0, pe1, pe2, mask, W_qkv, b_qkv):
    nc = _get_nc()
    in_maps = prep_inputs(x, pe0, pe1, pe2, mask, W_qkv, b_qkv)
    res = run_bass_kernel_spmd(nc, in_maps, core_ids=list(range(N_CORES)))
    return assemble_output(res.results)
